# revision 35
# baseline (speedup 1.0000x reference)
"""Trainium2 Bass kernel for nn_BasicLSTM: (B,T,N,C) shared-weight LSTM -> FC.

Strategy (data parallel over 8 cores, B=64 -> 8 batches/core). The kernel is
ACT(Scalar)-engine bound (5H LUT evals per seq-step), so the layout is built
around minimizing Scalar-engine time and keeping it bubble-free:

  - seqs = 8*1370 = 10960 sequences per core, T=12, C=8, H=64; 11 "pairs" of
    two 512-seq blocks (blk0 -> partitions 0:64, blk1 -> 64:128).
  - Per pair-step: 8 matmuls (4 gate sections x 2 blocks) with stationary
    lhsT = [W_hh.T; W_ih.T; b] (73 x 64) into ONE (128, 2048) PSUM tile with
    sections [i | g~ | o | f], where the g~ section's weights are pre-scaled
    by 2 on the host.
  - ONE Sigmoid over the whole (128, 2048) tile. tanh(g) is recovered on DVE
    as 2*sigmoid(2g)-1 via a single tensor_scalar (4x fast mode) -- this
    removes one of three ACT instructions per step.
  - tanh(c) is batched across 8 pairs (an octa shares a (128, 4096) c tile)
    and emitted ~2 sigmoids late, so ACT never waits on the DVE cell update.
  - Everything lives in fp16 (not bf16): keeps the DVE 2x fast mode and the
    10-bit mantissa avoids cancellation in 2*sigmoid-1 near 0.5.
  - h = sigmoid(o)*tanh(c) as one (128,512) DVE mult; two SBUF->SBUF DMAs
    scatter the halves into the next step's rhs tile [h(0:64); x;ones(64:73)].
  - All 11 pairs are processed round-robin in a single flat t-loop (no group
    boundaries); x arrives host-transposed as (9, T, seqs) fp16 with a ones
    channel carrying the biases; first-round x-DMAs go on the sync queue for
    a fast ramp, the rest stream on the gpsimd queue 2 steps ahead.
  - Tail: at t=11 each pair is flushed individually and y = W_fc h + b_fc is
    computed straight from the pair's h tile (bias via a second accumulating
    matmul against a ones row), so the last pair's chain is only a few us.
"""

from contextlib import ExitStack

import numpy as np

import concourse.bass as bass
import concourse.mybir as mybir
import concourse.tile as tile
from concourse import bacc
from concourse.bass_utils import run_bass_kernel_spmd
from concourse.tile import add_dep_helper

B, T, N, C, H = 64, 12, 1370, 8, 64
NCORES = 8
BPC = B // NCORES          # batches per core
SEQS = BPC * N             # 10960 sequences per core
S = 512                    # block size (free dim per matmul)
PAIRW = 2 * S              # pair width in rhs tiles
KH = H                     # 64 rows of h in rhs
KX = C + 1                 # 8 x-channels + ones row
K = KH + KX                # 73
G4 = 4 * H                 # 256

FP16 = mybir.dt.float16
F32 = mybir.dt.float32
NPFP16 = np.float16

AF = mybir.ActivationFunctionType
ALU = mybir.AluOpType


def _pairs(seqs: int, s: int = S):
    blocks = [(c0, min(s, seqs - c0)) for c0 in range(0, seqs, s)]
    assert len(blocks) % 2 == 0
    out = []
    for i in range(0, len(blocks), 2):
        (c0, w0), (c1, w1) = blocks[i], blocks[i + 1]
        assert c1 == c0 + w0 and w0 == s
        out.append((c0, w0, w1))
    return out


def build_nc() -> bass.Bass:
    nc = bacc.Bacc("TRN2", target_bir_lowering=False, debug=False)

    xin = nc.declare_dram_parameter("xin", [KX, T, SEQS], FP16, isOutput=False)
    wg = nc.declare_dram_parameter("wg", [K, G4], FP16, isOutput=False)
    wfc = nc.declare_dram_parameter("wfc", [H + 1, C], FP16, isOutput=False)
    bfc = nc.declare_dram_parameter("bfc", [C, 1], F32, isOutput=False)
    y = nc.declare_dram_parameter("y", [C, SEQS], F32, isOutput=True)

    pairs = _pairs(SEQS)
    NP = len(pairs)                       # 11
    # tanh(c) batches: one octa + the rest
    batches = [list(range(0, 8)), list(range(8, NP))]

    with tile.TileContext(nc) as tc, ExitStack() as ctx:
        const = ctx.enter_context(tc.tile_pool(name="const", bufs=1))
        rhsp = ctx.enter_context(tc.tile_pool(name="rhs", bufs=34))
        sgp = ctx.enter_context(tc.tile_pool(name="sg", bufs=13))
        thgp = ctx.enter_context(tc.tile_pool(name="thg", bufs=4))
        igp = ctx.enter_context(tc.tile_pool(name="ig", bufs=4))
        fcp = ctx.enter_context(tc.tile_pool(name="fcm", bufs=4))
        cbp = ctx.enter_context(tc.tile_pool(name="cb", bufs=len(batches)))
        thcp = ctx.enter_context(tc.tile_pool(name="thc", bufs=3))
        htp = ctx.enter_context(tc.tile_pool(name="ht", bufs=12))
        ysp = ctx.enter_context(tc.tile_pool(name="ys", bufs=3))
        gpp = ctx.enter_context(tc.tile_pool(name="gp", bufs=2, space="PSUM"))

        w_sb = const.tile([K, G4], FP16)
        nc.sync.dma_start(out=w_sb[:, :], in_=wg[:, :])
        # ones rows for the FC bias matmuls (also an early ACT table load)
        scratch = const.tile([128, S], FP16)
        nc.vector.memset(scratch[:, :], 1.0)
        # rows 0 and 64 stay 1.0 (FC bias ones rows); warm up ACT on row 32
        nc.scalar.activation(scratch[32:33, 0:8], scratch[32:33, 0:8], AF.Sigmoid)

        # gate section offsets in w_sb cols: [i, g~(x2), o, f]
        SECS = (0, H, 2 * H, 3 * H)

        st = [dict(rhs=[None] * T, sg=None) for _ in range(NP)]

        def alloc_rhs(p, t, queue):
            c0, w0, w1 = pairs[p]
            rt = rhsp.tile([K, PAIRW], FP16, name="rhs", tag="rhs")
            st[p]["rhs"][t] = rt
            queue.dma_start(out=rt[KH:K, 0 : w0 + w1], in_=xin[:, t, c0 : c0 + w0 + w1])

        for p in range(NP):
            alloc_rhs(p, 0, nc.sync)
        # W_fc.T duplicated in both partition halves + b_fc rows at partitions
        # 0 and 64, so FC matmuls have lhsT and rhs at matching partition base
        wfc_sb = const.tile([128, C], FP16)
        nc.sync.dma_start(out=wfc_sb[0:H, :], in_=wfc[0:H, :])
        nc.sync.dma_start(out=wfc_sb[H : 2 * H, :], in_=wfc[0:H, :])
        # b_fc as a per-partition scalar for the FC copy (partitions 0:8, 32:40)
        bfc_sb = const.tile([40, 1], F32)
        nc.vector.memset(bfc_sb[:, :], 0.0)
        nc.sync.dma_start(out=bfc_sb[0:C, 0:1], in_=bfc[:, :])
        nc.sync.dma_start(out=bfc_sb[32 : 32 + C, 0:1], in_=bfc[:, :])
        for p in range(NP):
            alloc_rhs(p, 1, nc.gpsimd)

        # persistent cell-state tiles, one per tanh batch (pairs side by side)
        cb = [cbp.tile([128, 8 * S], FP16, name=f"cb{b}", tag="cb")
              for b in range(len(batches))]
        cslot = {}
        for b, batch in enumerate(batches):
            for j, p in enumerate(batch):
                cslot[p] = (cb[b], j * S)

        def emit_fc(p, ht):
            """y = W_fc @ h + b_fc for both blocks of pair p, straight from
            the (128, S) h tile; bias via an accumulating matmul on a ones
            row. blk0 at PSUM partitions 0:8, blk1 at 32:40."""
            c0, w0, w1 = pairs[p]
            pf = gpp.tile([128, 4 * S], F32, name="gates", tag="gates")
            for blk, (wd, rbase, pos) in enumerate(((w0, 0, 0), (w1, 64, 32))):
                nc.tensor.matmul(
                    pf[pos : pos + C, 0:wd], wfc_sb[rbase : rbase + H, :],
                    ht[rbase : rbase + H, 0:wd],
                    tile_position=(rbase, pos),
                )
            ys = ysp.tile([40, S], F32, name="ys", tag="ys")
            # PSUM->SBUF copy fused with the +b_fc bias (per-partition scalar);
            # alternate between DVE and ACT so neither final-round queue
            # becomes the bottleneck
            if p % 2 == 0:
                nc.vector.tensor_scalar(
                    ys[:, :], pf[0:40, 0:S], bfc_sb[:, 0:1], None, ALU.add
                )
            else:
                nc.scalar.add(ys[:, :], pf[0:40, 0:S], bfc_sb[:, 0:1])
            nc.sync.dma_start(out=y[:, c0 : c0 + w0], in_=ys[0:C, 0:w0])
            nc.sync.dma_start(out=y[:, c0 + w0 : c0 + w0 + w1], in_=ys[32 : 32 + C, 0:w1])

        def emit_tanh_batch(b, after=None):
            """One tanh(c) instruction covering a whole batch of pairs."""
            nw = len(batches[b]) * S
            thc = thcp.tile([128, 8 * S], FP16, name="thc", tag="thc")
            th = nc.scalar.activation(thc[:, 0:nw], cb[b][:, 0:nw], AF.Tanh)
            if after is not None:
                # pin the tanh's Scalar-queue slot: its real dep (the last
                # cell add of the batch) retires late on DVE, and the
                # scheduler otherwise places this tanh early enough to
                # head-of-line block the Scalar engine on it
                add_dep_helper(th.ins, after.ins, reason="tanh placement")
            return thc

        def emit_hts(b, js, tq, thc):
            """h = sig(o)*tanh(c) for a subset of batch b's pairs, h halves
            scattered into each pair's t=tq+1 rhs tile. Only for tq < T-1."""
            for j in js:
                p = batches[b][j]
                c0, w0, w1 = pairs[p]
                sg = st[p]["sg"]
                ht = htp.tile([128, S], FP16, name="ht", tag="ht")
                nc.vector.tensor_mul(
                    ht[:, :], sg[:, 2 * S : 3 * S], thc[:, j * S : j * S + S]
                )
                rt = st[p]["rhs"][tq + 1]
                nc.sync.dma_start(out=rt[0:KH, 0:w0], in_=ht[0:KH, 0:w0])
                nc.sync.dma_start(out=rt[0:KH, w0 : w0 + w1], in_=ht[KH:128, 0:w1])

        def flush_final(plist):
            """t = T-1: tanh + h for 1-2 consecutive pairs sharing a c tile
            (FC emitted separately, lagged)."""
            ctile, coff = cslot[plist[0]]
            nw = len(plist) * S
            thc = thcp.tile([128, 8 * S], FP16, name="thc", tag="thc")
            nc.scalar.activation(thc[:, 0:nw], ctile[:, coff : coff + nw], AF.Tanh)
            for j, p in enumerate(plist):
                sg = st[p]["sg"]
                ht = htp.tile([128, S], FP16, name="ht", tag="ht")
                nc.vector.tensor_mul(
                    ht[:, :], sg[:, 2 * S : 3 * S], thc[:, j * S : j * S + S]
                )
                st[p]["ht"] = ht

        hold = {}
        for t in range(T):
            for p in range(NP):
                # delayed batch-1 tanh flush (never on the T-1 round's own
                # batches -- those flush per-pair below)
                if p == 4 and t > 0:
                    thc1 = emit_tanh_batch(1, after=st[3]["sig"])
                    emit_hts(1, range(len(batches[1])), t - 1, thc1)
                if t + 2 < T:
                    alloc_rhs(p, t + 2, nc.gpsimd)
                c0, w0, w1 = pairs[p]
                rt = st[p]["rhs"][t]
                gates = gpp.tile([128, 4 * S], F32, name="gates", tag="gates")
                use = SECS if t > 0 else SECS[:3]   # no f gate at t=0 (c=0)
                for si, gof in enumerate(use):
                    for blk, (cbs, wd) in enumerate(((0, w0), (w0, w1))):
                        pb = 64 * blk
                        if t == 0:
                            lh = w_sb[KH:K, gof : gof + H]
                            rh = rt[KH:K, cbs : cbs + wd]
                        else:
                            lh = w_sb[:, gof : gof + H]
                            rh = rt[:, cbs : cbs + wd]
                        nc.tensor.matmul(
                            gates[pb : pb + 64, si * S : si * S + wd], lh, rh
                        )
                nsec = len(use)
                sg = sgp.tile([128, 4 * S], FP16, name="sg", tag="sg")
                st[p]["sig"] = nc.scalar.activation(
                    sg[:, 0 : nsec * S], gates[:, 0 : nsec * S], AF.Sigmoid
                )
                st[p]["sg"] = sg
                # tanh(g) = 2*sigmoid(2g) - 1 (x2 baked into the weights)
                thg = thgp.tile([128, S], FP16, name="thg", tag="thg")
                nc.vector.tensor_scalar(
                    thg[:, :], sg[:, S : 2 * S], 2.0, -1.0, ALU.mult, ALU.add
                )
                ctile, coff = cslot[p]
                cap = ctile[:, coff : coff + S]
                if t == 0:
                    nc.vector.tensor_mul(cap, sg[:, 0:S], thg[:, :])
                else:
                    ig = igp.tile([128, S], FP16, name="ig", tag="ig")
                    nc.vector.tensor_mul(ig[:, :], sg[:, 0:S], thg[:, :])
                    fcm = fcp.tile([128, S], FP16, name="fcm", tag="fcm")
                    nc.vector.tensor_mul(fcm[:, :], sg[:, 3 * S : 4 * S], cap)
                    nc.vector.tensor_add(cap, ig[:, :], fcm[:, :])
                if t == T - 1:
                    # lag the final tanh+h (2-pair batches) and the FC by 4
                    # pairs so its matmuls never block the PE queue
                    if p >= 2 and p % 2 == 0:
                        flush_final([p - 2, p - 1])
                    if p >= 4:
                        emit_fc(p - 4, st[p - 4]["ht"])
                elif p == 8:
                    # tanh(b0) as early as its deps allow (adds of p0..p7),
                    # so pair 0's h round-trip clears before the next round
                    hold["thc0"] = emit_tanh_batch(0, after=st[8]["sig"])
                elif p == 9:
                    # split batch-0's h burst around pair 10's DVE chain so
                    # pair 10's cell add retires early (the b1 tanh needs it)
                    emit_hts(0, (0, 1, 2, 3), t, hold["thc0"])
                elif p == 10:
                    emit_hts(0, (4, 5, 6, 7), t, hold["thc0"])
        flush_final([NP - 1])
        for p in range(NP - 4, NP):
            emit_fc(p, st[p]["ht"])

    nc.compile()
    return nc


def prep_inputs(x, W_ih, W_hh, b_ih, b_hh, W_fc, b_fc, seqs=SEQS, ncores=NCORES):
    """Host-side shard + transpose + weight packing. Returns in_maps."""
    x = np.asarray(x, dtype=np.float32)
    W_ih = np.asarray(W_ih, dtype=np.float32)
    W_hh = np.asarray(W_hh, dtype=np.float32)
    b = np.asarray(b_ih, dtype=np.float32) + np.asarray(b_hh, dtype=np.float32)
    W_fc = np.asarray(W_fc, dtype=np.float32)
    b_fc = np.asarray(b_fc, dtype=np.float32)

    # sections [i, g~(x2), o, f] <- pytorch row groups (i, f, g, o)
    wg = np.zeros((K, G4), dtype=np.float32)
    for dst, src, scale in ((0, 0, 1.0), (H, 2 * H, 2.0), (2 * H, 3 * H, 1.0),
                            (3 * H, H, 1.0)):
        rows = slice(src, src + H)
        wg[0:KH, dst : dst + H] = scale * W_hh[rows, :].T
        wg[KH : KH + C, dst : dst + H] = scale * W_ih[rows, :].T
        wg[K - 1, dst : dst + H] = scale * b[rows]
    wg = wg.astype(NPFP16)

    wfc = np.concatenate([W_fc.T, b_fc[None, :]], axis=0).astype(NPFP16)  # (65, 8)
    bfc = b_fc.reshape(C, 1).astype(np.float32)

    bpc = x.shape[0] // ncores
    in_maps = []
    for k in range(ncores):
        xc = x[k * bpc : (k + 1) * bpc]              # (bpc, T, N, C)
        xt = xc.transpose(3, 1, 0, 2).reshape(C, T, seqs)
        xext = np.empty((KX, T, seqs), dtype=NPFP16)
        xext[0:C] = xt.astype(NPFP16)
        xext[C] = np.ones((T, seqs), dtype=NPFP16)
        in_maps.append({"xin": xext, "wg": wg, "wfc": wfc, "bfc": bfc})
    return in_maps


_CACHE = {}


def _get_nc():
    if "nc" not in _CACHE:
        _CACHE["nc"] = build_nc()
    return _CACHE["nc"]


def kernel(x, W_ih, W_hh, b_ih, b_hh, W_fc, b_fc, **run_kwargs):
    nc = _get_nc()
    in_maps = prep_inputs(x, W_ih, W_hh, b_ih, b_hh, W_fc, b_fc)
    res = run_bass_kernel_spmd(nc, in_maps, list(range(NCORES)), **run_kwargs)
    outs = res.results
    ys = []
    for k in range(NCORES):
        yk = np.asarray(outs[k]["y"])               # (C, SEQS) f32
        ys.append(yk.T.reshape(BPC, N, C))
    y = np.concatenate(ys, axis=0)                  # (B, N, C)
    if run_kwargs.get("trace"):
        _CACHE["last_result"] = res
    return y.astype(np.float32)


# revision 42
# speedup vs baseline: 1.1122x; 1.1122x over previous
"""Trainium2 Bass kernel for nn_BasicLSTM: (B,T,N,C) shared-weight LSTM -> FC.

Strategy (data parallel over 8 cores, B=64 -> 8 batches/core). The kernel is
ACT(Scalar)-engine bound (5H LUT evals per seq-step), so the layout is built
around minimizing Scalar-engine time and keeping it bubble-free:

  - seqs = 8*1370 = 10960 sequences per core, T=12, C=8, H=64; 11 "pairs" of
    two 512-seq blocks (blk0 -> partitions 0:64, blk1 -> 64:128).
  - Per pair-step: 8 matmuls (4 gate sections x 2 blocks) with stationary
    lhsT = [W_hh.T; W_ih.T; b] (73 x 64) into ONE (128, 2048) PSUM tile with
    sections [i | g~ | o | f], where the g~ section's weights are pre-scaled
    by 2 on the host.
  - ONE Sigmoid over the whole (128, 2048) tile. tanh(g) is recovered on DVE
    as 2*sigmoid(2g)-1 via a single tensor_scalar (4x fast mode) -- this
    removes one of three ACT instructions per step.
  - tanh(c) is batched across 8 pairs (an octa shares a (128, 4096) c tile)
    and emitted ~2 sigmoids late, so ACT never waits on the DVE cell update.
  - Everything lives in fp16 (not bf16): keeps the DVE 2x fast mode and the
    10-bit mantissa avoids cancellation in 2*sigmoid-1 near 0.5.
  - h = sigmoid(o)*tanh(c) as one (128,512) DVE mult; two SBUF->SBUF DMAs
    scatter the halves into the next step's rhs tile [h(0:64); x;ones(64:73)].
  - All 11 pairs are processed round-robin in a single flat t-loop (no group
    boundaries); x arrives host-transposed as (9, T, seqs) fp16 with a ones
    channel carrying the biases; first-round x-DMAs go on the sync queue for
    a fast ramp, the rest stream on the gpsimd queue 2 steps ahead.
  - Tail: at t=11 each pair is flushed individually and y = W_fc h + b_fc is
    computed straight from the pair's h tile (bias via a second accumulating
    matmul against a ones row), so the last pair's chain is only a few us.
"""

from contextlib import ExitStack

import numpy as np

import concourse.bass as bass
import concourse.mybir as mybir
import concourse.tile as tile
from concourse import bacc
from concourse.bass_utils import run_bass_kernel_spmd
from concourse.tile import add_dep_helper

B, T, N, C, H = 64, 12, 1370, 8, 64
NCORES = 8
BPC = B // NCORES          # batches per core
SEQS = BPC * N             # 10960 sequences per core
S = 512                    # block size (free dim per matmul)
PAIRW = 2 * S              # pair width in rhs tiles
KH = H                     # 64 rows of h in rhs
KX = C + 1                 # 8 x-channels + ones row
K = KH + KX                # 73
G4 = 4 * H                 # 256

FP16 = mybir.dt.float16
F32 = mybir.dt.float32
NPFP16 = np.float16

AF = mybir.ActivationFunctionType
ALU = mybir.AluOpType


def _pairs(seqs: int, s: int = S):
    blocks = [(c0, min(s, seqs - c0)) for c0 in range(0, seqs, s)]
    assert len(blocks) % 2 == 0
    out = []
    for i in range(0, len(blocks), 2):
        (c0, w0), (c1, w1) = blocks[i], blocks[i + 1]
        assert c1 == c0 + w0 and w0 == s
        out.append((c0, w0, w1))
    return out


def build_nc() -> bass.Bass:
    nc = bacc.Bacc("TRN2", target_bir_lowering=False, debug=False)

    xin = nc.declare_dram_parameter("xin", [KX, T, SEQS], FP16, isOutput=False)
    wg = nc.declare_dram_parameter("wg", [K, G4], FP16, isOutput=False)
    wfc = nc.declare_dram_parameter("wfc", [H + 1, C], FP16, isOutput=False)
    bfc = nc.declare_dram_parameter("bfc", [C, 1], F32, isOutput=False)
    y = nc.declare_dram_parameter("y", [C, SEQS], F32, isOutput=True)

    pairs = _pairs(SEQS)
    NP = len(pairs)                       # 11
    # tanh(c) batches: two quads + a triple
    batches = [[0, 1, 2, 3], [4, 5, 6, 7], [8, 9, 10]]

    with tile.TileContext(nc) as tc, ExitStack() as ctx:
        const = ctx.enter_context(tc.tile_pool(name="const", bufs=1))
        rhsp = ctx.enter_context(tc.tile_pool(name="rhs", bufs=34))
        sgp = ctx.enter_context(tc.tile_pool(name="sg", bufs=13))
        thgp = ctx.enter_context(tc.tile_pool(name="thg", bufs=4))
        igp = ctx.enter_context(tc.tile_pool(name="ig", bufs=4))
        fcp = ctx.enter_context(tc.tile_pool(name="fcm", bufs=4))
        cbp = ctx.enter_context(tc.tile_pool(name="cb", bufs=len(batches)))
        thcp = ctx.enter_context(tc.tile_pool(name="thc", bufs=3))
        htp = ctx.enter_context(tc.tile_pool(name="ht", bufs=12))
        ysp = ctx.enter_context(tc.tile_pool(name="ys", bufs=3))
        gpp = ctx.enter_context(tc.tile_pool(name="gp", bufs=2, space="PSUM"))

        w_sb = const.tile([K, G4], FP16)
        nc.sync.dma_start(out=w_sb[:, :], in_=wg[:, :])
        # ones rows for the FC bias matmuls (also an early ACT table load)
        scratch = const.tile([128, S], FP16)
        nc.vector.memset(scratch[:, :], 1.0)
        # rows 0 and 64 stay 1.0 (FC bias ones rows); warm up ACT on row 32
        nc.scalar.activation(scratch[32:33, 0:8], scratch[32:33, 0:8], AF.Sigmoid)

        # gate section offsets in w_sb cols: [i, g~(x2), o, f]
        SECS = (0, H, 2 * H, 3 * H)

        st = [dict(rhs=[None] * T, sg=None) for _ in range(NP)]

        def alloc_rhs(p, t, queue):
            c0, w0, w1 = pairs[p]
            rt = rhsp.tile([K, PAIRW], FP16, name="rhs", tag="rhs")
            st[p]["rhs"][t] = rt
            queue.dma_start(out=rt[KH:K, 0 : w0 + w1], in_=xin[:, t, c0 : c0 + w0 + w1])

        for p in range(NP):
            alloc_rhs(p, 0, nc.sync)
        # W_fc.T duplicated in both partition halves + b_fc rows at partitions
        # 0 and 64, so FC matmuls have lhsT and rhs at matching partition base
        wfc_sb = const.tile([128, C], FP16)
        nc.sync.dma_start(out=wfc_sb[0:H, :], in_=wfc[0:H, :])
        nc.sync.dma_start(out=wfc_sb[H : 2 * H, :], in_=wfc[0:H, :])
        # b_fc as a per-partition scalar for the FC copy (partitions 0:8, 32:40)
        bfc_sb = const.tile([40, 1], F32)
        nc.vector.memset(bfc_sb[:, :], 0.0)
        nc.sync.dma_start(out=bfc_sb[0:C, 0:1], in_=bfc[:, :])
        nc.sync.dma_start(out=bfc_sb[32 : 32 + C, 0:1], in_=bfc[:, :])
        for p in range(NP):
            alloc_rhs(p, 1, nc.gpsimd)

        # persistent cell-state tiles, one per tanh batch (pairs side by side)
        cb = [cbp.tile([128, 4 * S], FP16, name=f"cb{b}", tag="cb")
              for b in range(len(batches))]
        cslot = {}
        for b, batch in enumerate(batches):
            for j, p in enumerate(batch):
                cslot[p] = (cb[b], j * S)

        def emit_fc(p, ht):
            """y = W_fc @ h + b_fc for both blocks of pair p, straight from
            the (128, S) h tile; bias via an accumulating matmul on a ones
            row. blk0 at PSUM partitions 0:8, blk1 at 32:40."""
            c0, w0, w1 = pairs[p]
            pf = gpp.tile([128, 4 * S], F32, name="gates", tag="gates")
            for blk, (wd, rbase, pos) in enumerate(((w0, 0, 0), (w1, 64, 32))):
                nc.tensor.matmul(
                    pf[pos : pos + C, 0:wd], wfc_sb[rbase : rbase + H, :],
                    ht[rbase : rbase + H, 0:wd],
                    tile_position=(rbase, pos),
                )
            ys = ysp.tile([40, S], F32, name="ys", tag="ys")
            # PSUM->SBUF copy fused with the +b_fc bias (per-partition scalar);
            # alternate between DVE and ACT so neither final-round queue
            # becomes the bottleneck
            if p % 2 == 0:
                nc.vector.tensor_scalar(
                    ys[:, :], pf[0:40, 0:S], bfc_sb[:, 0:1], None, ALU.add
                )
            else:
                nc.scalar.add(ys[:, :], pf[0:40, 0:S], bfc_sb[:, 0:1])
            nc.sync.dma_start(out=y[:, c0 : c0 + w0], in_=ys[0:C, 0:w0])
            nc.sync.dma_start(out=y[:, c0 + w0 : c0 + w0 + w1], in_=ys[32 : 32 + C, 0:w1])

        def emit_tanh_batch(b, after=None):
            """One tanh(c) instruction covering a whole batch of pairs."""
            nw = len(batches[b]) * S
            thc = thcp.tile([128, 4 * S], FP16, name="thc", tag="thc")
            th = nc.scalar.activation(thc[:, 0:nw], cb[b][:, 0:nw], AF.Tanh)
            if after is not None:
                # pin the tanh's Scalar-queue slot: its real dep (the last
                # cell add of the batch) retires late on DVE, and the
                # scheduler otherwise places this tanh early enough to
                # head-of-line block the Scalar engine on it
                add_dep_helper(th.ins, after.ins, reason="tanh placement")
            return thc

        def emit_hts(b, js, tq, thc):
            """h = sig(o)*tanh(c) for a subset of batch b's pairs, h halves
            scattered into each pair's t=tq+1 rhs tile. Only for tq < T-1."""
            for j in js:
                p = batches[b][j]
                c0, w0, w1 = pairs[p]
                sg = st[p]["sg"]
                ht = htp.tile([128, S], FP16, name="ht", tag="ht")
                nc.vector.tensor_mul(
                    ht[:, :], sg[:, 2 * S : 3 * S], thc[:, j * S : j * S + S]
                )
                rt = st[p]["rhs"][tq + 1]
                nc.sync.dma_start(out=rt[0:KH, 0:w0], in_=ht[0:KH, 0:w0])
                nc.sync.dma_start(out=rt[0:KH, w0 : w0 + w1], in_=ht[KH:128, 0:w1])

        def flush_final(plist):
            """t = T-1: tanh + h for 1-2 consecutive pairs sharing a c tile
            (FC emitted separately, lagged)."""
            ctile, coff = cslot[plist[0]]
            nw = len(plist) * S
            thc = thcp.tile([128, 4 * S], FP16, name="thc", tag="thc")
            nc.scalar.activation(thc[:, 0:nw], ctile[:, coff : coff + nw], AF.Tanh)
            for j, p in enumerate(plist):
                sg = st[p]["sg"]
                ht = htp.tile([128, S], FP16, name="ht", tag="ht")
                nc.vector.tensor_mul(
                    ht[:, :], sg[:, 2 * S : 3 * S], thc[:, j * S : j * S + S]
                )
                st[p]["ht"] = ht

        hold = {}
        for t in range(T):
            for p in range(NP):
                # delayed batch-2 tanh flush (never on the T-1 round's own
                # batches -- those flush per-pair below)
                if p == 4 and t > 0:
                    thc2 = emit_tanh_batch(2, after=st[3]["sig"])
                    emit_hts(2, range(len(batches[2])), t - 1, thc2)
                if t + 2 < T:
                    alloc_rhs(p, t + 2, nc.gpsimd)
                c0, w0, w1 = pairs[p]
                rt = st[p]["rhs"][t]
                gates = gpp.tile([128, 4 * S], F32, name="gates", tag="gates")
                use = SECS if t > 0 else SECS[:3]   # no f gate at t=0 (c=0)
                for si, gof in enumerate(use):
                    for blk, (cbs, wd) in enumerate(((0, w0), (w0, w1))):
                        pb = 64 * blk
                        if t == 0:
                            lh = w_sb[KH:K, gof : gof + H]
                            rh = rt[KH:K, cbs : cbs + wd]
                        else:
                            lh = w_sb[:, gof : gof + H]
                            rh = rt[:, cbs : cbs + wd]
                        nc.tensor.matmul(
                            gates[pb : pb + 64, si * S : si * S + wd], lh, rh
                        )
                nsec = len(use)
                sg = sgp.tile([128, 4 * S], FP16, name="sg", tag="sg")
                st[p]["sig"] = nc.scalar.activation(
                    sg[:, 0 : nsec * S], gates[:, 0 : nsec * S], AF.Sigmoid
                )
                st[p]["sg"] = sg
                # tanh(g) = 2*sigmoid(2g) - 1 (x2 baked into the weights)
                thg = thgp.tile([128, S], FP16, name="thg", tag="thg")
                nc.vector.tensor_scalar(
                    thg[:, :], sg[:, S : 2 * S], 2.0, -1.0, ALU.mult, ALU.add
                )
                ctile, coff = cslot[p]
                cap = ctile[:, coff : coff + S]
                if t == 0:
                    nc.vector.tensor_mul(cap, sg[:, 0:S], thg[:, :])
                else:
                    ig = igp.tile([128, S], FP16, name="ig", tag="ig")
                    nc.vector.tensor_mul(ig[:, :], sg[:, 0:S], thg[:, :])
                    fcm = fcp.tile([128, S], FP16, name="fcm", tag="fcm")
                    nc.vector.tensor_mul(fcm[:, :], sg[:, 3 * S : 4 * S], cap)
                    nc.vector.tensor_add(cap, ig[:, :], fcm[:, :])
                if t == T - 1:
                    # lag the final tanh+h (2-pair batches) and the FC by 4
                    # pairs so its matmuls never block the PE queue
                    if p >= 2 and p % 2 == 0:
                        flush_final([p - 2, p - 1])
                    if p >= 4:
                        emit_fc(p - 4, st[p - 4]["ht"])
                elif p == 6:
                    # batch-0's deps (adds of p0..p3) are ready by now; an
                    # early flush lands p0/p1's h well before the next round
                    hold["thc0"] = emit_tanh_batch(0)
                    emit_hts(0, (0, 1, 2, 3), t, hold["thc0"])
                elif p == 9:
                    # split batch-1's h burst around pair 10's DVE chain so
                    # pair 10's cell add retires early (the b2 tanh needs it)
                    hold["thc1"] = emit_tanh_batch(1)
                    emit_hts(1, (0, 1), t, hold["thc1"])
                elif p == 10:
                    emit_hts(1, (2, 3), t, hold["thc1"])
        flush_final([NP - 1])
        for p in range(NP - 4, NP):
            emit_fc(p, st[p]["ht"])

    nc.compile()
    return nc


def prep_inputs(x, W_ih, W_hh, b_ih, b_hh, W_fc, b_fc, seqs=SEQS, ncores=NCORES):
    """Host-side shard + transpose + weight packing. Returns in_maps."""
    x = np.asarray(x, dtype=np.float32)
    W_ih = np.asarray(W_ih, dtype=np.float32)
    W_hh = np.asarray(W_hh, dtype=np.float32)
    b = np.asarray(b_ih, dtype=np.float32) + np.asarray(b_hh, dtype=np.float32)
    W_fc = np.asarray(W_fc, dtype=np.float32)
    b_fc = np.asarray(b_fc, dtype=np.float32)

    # sections [i, g~(x2), o, f] <- pytorch row groups (i, f, g, o)
    wg = np.zeros((K, G4), dtype=np.float32)
    for dst, src, scale in ((0, 0, 1.0), (H, 2 * H, 2.0), (2 * H, 3 * H, 1.0),
                            (3 * H, H, 1.0)):
        rows = slice(src, src + H)
        wg[0:KH, dst : dst + H] = scale * W_hh[rows, :].T
        wg[KH : KH + C, dst : dst + H] = scale * W_ih[rows, :].T
        wg[K - 1, dst : dst + H] = scale * b[rows]
    wg = wg.astype(NPFP16)

    wfc = np.concatenate([W_fc.T, b_fc[None, :]], axis=0).astype(NPFP16)  # (65, 8)
    bfc = b_fc.reshape(C, 1).astype(np.float32)

    bpc = x.shape[0] // ncores
    in_maps = []
    for k in range(ncores):
        xc = x[k * bpc : (k + 1) * bpc]              # (bpc, T, N, C)
        xt = xc.transpose(3, 1, 0, 2).reshape(C, T, seqs)
        xext = np.empty((KX, T, seqs), dtype=NPFP16)
        xext[0:C] = xt.astype(NPFP16)
        xext[C] = np.ones((T, seqs), dtype=NPFP16)
        in_maps.append({"xin": xext, "wg": wg, "wfc": wfc, "bfc": bfc})
    return in_maps


_CACHE = {}


def _get_nc():
    if "nc" not in _CACHE:
        _CACHE["nc"] = build_nc()
    return _CACHE["nc"]


def kernel(x, W_ih, W_hh, b_ih, b_hh, W_fc, b_fc, **run_kwargs):
    nc = _get_nc()
    in_maps = prep_inputs(x, W_ih, W_hh, b_ih, b_hh, W_fc, b_fc)
    res = run_bass_kernel_spmd(nc, in_maps, list(range(NCORES)), **run_kwargs)
    outs = res.results
    ys = []
    for k in range(NCORES):
        yk = np.asarray(outs[k]["y"])               # (C, SEQS) f32
        ys.append(yk.T.reshape(BPC, N, C))
    y = np.concatenate(ys, axis=0)                  # (B, N, C)
    if run_kwargs.get("trace"):
        _CACHE["last_result"] = res
    return y.astype(np.float32)


# revision 46
# speedup vs baseline: 1.1164x; 1.0038x over previous
"""Trainium2 Bass kernel for nn_BasicLSTM: (B,T,N,C) shared-weight LSTM -> FC.

Strategy (data parallel over 8 cores, B=64 -> 8 batches/core). The kernel is
ACT(Scalar)-engine bound (5H LUT evals per seq-step), so the layout is built
around minimizing Scalar-engine time and keeping it bubble-free:

  - seqs = 8*1370 = 10960 sequences per core, T=12, C=8, H=64; 11 "pairs" of
    two 512-seq blocks (blk0 -> partitions 0:64, blk1 -> 64:128).
  - Per pair-step: 8 matmuls (4 gate sections x 2 blocks) with stationary
    lhsT = [W_hh.T; W_ih.T; b] (73 x 64) into ONE (128, 2048) PSUM tile with
    sections [i | g~ | o | f], where the g~ section's weights are pre-scaled
    by 2 on the host.
  - ONE Sigmoid over the whole (128, 2048) tile. tanh(g) is recovered on DVE
    as 2*sigmoid(2g)-1 via a single tensor_scalar (4x fast mode) -- this
    removes one of three ACT instructions per step.
  - tanh(c) is batched across 8 pairs (an octa shares a (128, 4096) c tile)
    and emitted ~2 sigmoids late, so ACT never waits on the DVE cell update.
  - Everything lives in fp16 (not bf16): keeps the DVE 2x fast mode and the
    10-bit mantissa avoids cancellation in 2*sigmoid-1 near 0.5.
  - h = sigmoid(o)*tanh(c) as one (128,512) DVE mult; two SBUF->SBUF DMAs
    scatter the halves into the next step's rhs tile [h(0:64); x;ones(64:73)].
  - All 11 pairs are processed round-robin in a single flat t-loop (no group
    boundaries); x arrives host-transposed as (9, T, seqs) fp16 with a ones
    channel carrying the biases; first-round x-DMAs go on the sync queue for
    a fast ramp, the rest stream on the gpsimd queue 2 steps ahead.
  - Tail: at t=11 each pair is flushed individually and y = W_fc h + b_fc is
    computed straight from the pair's h tile (bias via a second accumulating
    matmul against a ones row), so the last pair's chain is only a few us.
"""

from contextlib import ExitStack

import numpy as np

import concourse.bass as bass
import concourse.mybir as mybir
import concourse.tile as tile
from concourse import bacc
from concourse.bass_utils import run_bass_kernel_spmd
from concourse.tile import add_dep_helper

B, T, N, C, H = 64, 12, 1370, 8, 64
NCORES = 8
BPC = B // NCORES          # batches per core
SEQS = BPC * N             # 10960 sequences per core
S = 512                    # block size (free dim per matmul)
PAIRW = 2 * S              # pair width in rhs tiles
KH = H                     # 64 rows of h in rhs
KX = C + 1                 # 8 x-channels + ones row
K = KH + KX                # 73
G4 = 4 * H                 # 256

FP16 = mybir.dt.float16
F32 = mybir.dt.float32
NPFP16 = np.float16

AF = mybir.ActivationFunctionType
ALU = mybir.AluOpType


def _pairs(seqs: int, s: int = S):
    blocks = [(c0, min(s, seqs - c0)) for c0 in range(0, seqs, s)]
    assert len(blocks) % 2 == 0
    out = []
    for i in range(0, len(blocks), 2):
        (c0, w0), (c1, w1) = blocks[i], blocks[i + 1]
        assert c1 == c0 + w0 and w0 == s
        out.append((c0, w0, w1))
    return out


def build_nc() -> bass.Bass:
    nc = bacc.Bacc("TRN2", target_bir_lowering=False, debug=False)

    xin = nc.declare_dram_parameter("xin", [KX, T, SEQS], FP16, isOutput=False)
    wg = nc.declare_dram_parameter("wg", [K, G4], FP16, isOutput=False)
    wfc = nc.declare_dram_parameter("wfc", [H + 1, C], FP16, isOutput=False)
    bfc = nc.declare_dram_parameter("bfc", [C, 1], F32, isOutput=False)
    y = nc.declare_dram_parameter("y", [C, SEQS], F32, isOutput=True)

    pairs = _pairs(SEQS)
    NP = len(pairs)                       # 11
    # tanh(c) batches: two quads + a triple
    batches = [[0, 1, 2, 3], [4, 5, 6, 7], [8, 9, 10]]

    with tile.TileContext(nc) as tc, ExitStack() as ctx:
        const = ctx.enter_context(tc.tile_pool(name="const", bufs=1))
        rhsp = ctx.enter_context(tc.tile_pool(name="rhs", bufs=34))
        sgp = ctx.enter_context(tc.tile_pool(name="sg", bufs=13))
        thgp = ctx.enter_context(tc.tile_pool(name="thg", bufs=4))
        igp = ctx.enter_context(tc.tile_pool(name="ig", bufs=4))
        fcp = ctx.enter_context(tc.tile_pool(name="fcm", bufs=4))
        cbp = ctx.enter_context(tc.tile_pool(name="cb", bufs=len(batches)))
        thcp = ctx.enter_context(tc.tile_pool(name="thc", bufs=3))
        htp = ctx.enter_context(tc.tile_pool(name="ht", bufs=12))
        ysp = ctx.enter_context(tc.tile_pool(name="ys", bufs=3))
        gpp = ctx.enter_context(tc.tile_pool(name="gp", bufs=2, space="PSUM"))

        w_sb = const.tile([K, G4], FP16)
        nc.sync.dma_start(out=w_sb[:, :], in_=wg[:, :])
        # ones rows for the FC bias matmuls (also an early ACT table load)
        scratch = const.tile([128, S], FP16)
        nc.vector.memset(scratch[:, :], 1.0)
        # rows 0 and 64 stay 1.0 (FC bias ones rows); warm up ACT on row 32
        nc.scalar.activation(scratch[32:33, 0:8], scratch[32:33, 0:8], AF.Sigmoid)

        # gate section offsets in w_sb cols: [i, g~(x2), o, f]
        SECS = (0, H, 2 * H, 3 * H)

        st = [dict(rhs=[None] * T, sg=None) for _ in range(NP)]

        def alloc_rhs(p, t, queue):
            c0, w0, w1 = pairs[p]
            rt = rhsp.tile([K, PAIRW], FP16, name="rhs", tag="rhs")
            st[p]["rhs"][t] = rt
            queue.dma_start(out=rt[KH:K, 0 : w0 + w1], in_=xin[:, t, c0 : c0 + w0 + w1])

        for p in range(NP):
            alloc_rhs(p, 0, nc.sync)
        # W_fc.T duplicated in both partition halves + b_fc rows at partitions
        # 0 and 64, so FC matmuls have lhsT and rhs at matching partition base
        wfc_sb = const.tile([128, C], FP16)
        nc.sync.dma_start(out=wfc_sb[0:H, :], in_=wfc[0:H, :])
        nc.sync.dma_start(out=wfc_sb[H : 2 * H, :], in_=wfc[0:H, :])
        # b_fc as a per-partition scalar for the FC copy (partitions 0:8, 32:40)
        bfc_sb = const.tile([40, 1], F32)
        nc.vector.memset(bfc_sb[:, :], 0.0)
        nc.sync.dma_start(out=bfc_sb[0:C, 0:1], in_=bfc[:, :])
        nc.sync.dma_start(out=bfc_sb[32 : 32 + C, 0:1], in_=bfc[:, :])
        for p in range(NP):
            alloc_rhs(p, 1, nc.gpsimd)

        # persistent cell-state tiles, one per tanh batch (pairs side by side)
        cb = [cbp.tile([128, 4 * S], FP16, name=f"cb{b}", tag="cb")
              for b in range(len(batches))]
        cslot = {}
        for b, batch in enumerate(batches):
            for j, p in enumerate(batch):
                cslot[p] = (cb[b], j * S)

        def emit_fc(p, ht, copy_on_act=False):
            """y = W_fc @ h + b_fc for both blocks of pair p, straight from
            the (128, S) h tile. blk0 at PSUM partitions 0:8, blk1 at 32:40;
            the PSUM->SBUF copy is fused with the +b_fc bias (per-partition
            scalar)."""
            c0, w0, w1 = pairs[p]
            pf = gpp.tile([128, 4 * S], F32, name="gates", tag="gates")
            for blk, (wd, rbase, pos) in enumerate(((w0, 0, 0), (w1, 64, 32))):
                nc.tensor.matmul(
                    pf[pos : pos + C, 0:wd], wfc_sb[rbase : rbase + H, :],
                    ht[rbase : rbase + H, 0:wd],
                    tile_position=(rbase, pos),
                )
            ys = ysp.tile([40, S], F32, name="ys", tag="ys")
            if copy_on_act:
                nc.scalar.add(ys[:, :], pf[0:40, 0:S], bfc_sb[:, 0:1])
            else:
                nc.vector.tensor_scalar(
                    ys[:, :], pf[0:40, 0:S], bfc_sb[:, 0:1], None, ALU.add
                )
            nc.sync.dma_start(out=y[:, c0 : c0 + w0], in_=ys[0:C, 0:w0])
            nc.sync.dma_start(out=y[:, c0 + w0 : c0 + w0 + w1], in_=ys[32 : 32 + C, 0:w1])

        def emit_tanh_batch(b, after=None):
            """One tanh(c) instruction covering a whole batch of pairs."""
            nw = len(batches[b]) * S
            thc = thcp.tile([128, 4 * S], FP16, name="thc", tag="thc")
            th = nc.scalar.activation(thc[:, 0:nw], cb[b][:, 0:nw], AF.Tanh)
            if after is not None:
                # pin the tanh's Scalar-queue slot: its real dep (the last
                # cell add of the batch) retires late on DVE, and the
                # scheduler otherwise places this tanh early enough to
                # head-of-line block the Scalar engine on it
                add_dep_helper(th.ins, after.ins, reason="tanh placement")
            return thc

        def emit_hts(b, js, tq, thc):
            """h = sig(o)*tanh(c) for a subset of batch b's pairs, h halves
            scattered into each pair's t=tq+1 rhs tile. Only for tq < T-1."""
            for j in js:
                p = batches[b][j]
                c0, w0, w1 = pairs[p]
                sg = st[p]["sg"]
                ht = htp.tile([128, S], FP16, name="ht", tag="ht")
                nc.vector.tensor_mul(
                    ht[:, :], sg[:, 2 * S : 3 * S], thc[:, j * S : j * S + S]
                )
                rt = st[p]["rhs"][tq + 1]
                nc.sync.dma_start(out=rt[0:KH, 0:w0], in_=ht[0:KH, 0:w0])
                nc.sync.dma_start(out=rt[0:KH, w0 : w0 + w1], in_=ht[KH:128, 0:w1])

        def flush_final(plist, after=None):
            """t = T-1: tanh + h for 1-2 consecutive pairs sharing a c tile
            (FC emitted separately, lagged)."""
            ctile, coff = cslot[plist[0]]
            nw = len(plist) * S
            thc = thcp.tile([128, 4 * S], FP16, name="thc", tag="thc")
            th = nc.scalar.activation(thc[:, 0:nw], ctile[:, coff : coff + nw], AF.Tanh)
            if after is not None:
                add_dep_helper(th.ins, after.ins, reason="tanh placement")
            for j, p in enumerate(plist):
                sg = st[p]["sg"]
                ht = htp.tile([128, S], FP16, name="ht", tag="ht")
                nc.vector.tensor_mul(
                    ht[:, :], sg[:, 2 * S : 3 * S], thc[:, j * S : j * S + S]
                )
                st[p]["ht"] = ht

        hold = {}
        for t in range(T):
            for p in range(NP):
                # delayed batch-2 tanh flush (never on the T-1 round's own
                # batches -- those flush per-pair below)
                if p == 4 and t > 0:
                    thc2 = emit_tanh_batch(2, after=st[3]["sig"])
                    emit_hts(2, range(len(batches[2])), t - 1, thc2)
                if t + 2 < T:
                    alloc_rhs(p, t + 2, nc.gpsimd)
                c0, w0, w1 = pairs[p]
                rt = st[p]["rhs"][t]
                gates = gpp.tile([128, 4 * S], F32, name="gates", tag="gates")
                use = SECS if t > 0 else SECS[:3]   # no f gate at t=0 (c=0)
                for si, gof in enumerate(use):
                    for blk, (cbs, wd) in enumerate(((0, w0), (w0, w1))):
                        pb = 64 * blk
                        if t == 0:
                            lh = w_sb[KH:K, gof : gof + H]
                            rh = rt[KH:K, cbs : cbs + wd]
                        else:
                            lh = w_sb[:, gof : gof + H]
                            rh = rt[:, cbs : cbs + wd]
                        nc.tensor.matmul(
                            gates[pb : pb + 64, si * S : si * S + wd], lh, rh
                        )
                nsec = len(use)
                sg = sgp.tile([128, 4 * S], FP16, name="sg", tag="sg")
                st[p]["sig"] = nc.scalar.activation(
                    sg[:, 0 : nsec * S], gates[:, 0 : nsec * S], AF.Sigmoid
                )
                st[p]["sg"] = sg
                # tanh(g) = 2*sigmoid(2g) - 1 (x2 baked into the weights)
                thg = thgp.tile([128, S], FP16, name="thg", tag="thg")
                nc.vector.tensor_scalar(
                    thg[:, :], sg[:, S : 2 * S], 2.0, -1.0, ALU.mult, ALU.add
                )
                ctile, coff = cslot[p]
                cap = ctile[:, coff : coff + S]
                if t == 0:
                    nc.vector.tensor_mul(cap, sg[:, 0:S], thg[:, :])
                else:
                    ig = igp.tile([128, S], FP16, name="ig", tag="ig")
                    nc.vector.tensor_mul(ig[:, :], sg[:, 0:S], thg[:, :])
                    fcm = fcp.tile([128, S], FP16, name="fcm", tag="fcm")
                    nc.vector.tensor_mul(fcm[:, :], sg[:, 3 * S : 4 * S], cap)
                    nc.vector.tensor_add(cap, ig[:, :], fcm[:, :])
                if t == T - 1:
                    # lag the final tanh+h (2-pair batches) and the FC by 6
                    # pairs so its matmuls never crowd the PE queue
                    if p >= 2 and p % 2 == 0:
                        flush_final([p - 2, p - 1], after=st[p]["sig"])
                    if p >= 6:
                        emit_fc(p - 6, st[p - 6]["ht"])
                elif p == 6:
                    # batch-0's deps (adds of p0..p3) are ready by now; an
                    # early flush lands p0/p1's h well before the next round
                    hold["thc0"] = emit_tanh_batch(0)
                    emit_hts(0, (0, 1, 2, 3), t, hold["thc0"])
                elif p == 9:
                    # split batch-1's h burst around pair 10's DVE chain so
                    # pair 10's cell add retires early (the b2 tanh needs it)
                    hold["thc1"] = emit_tanh_batch(1)
                    emit_hts(1, (0, 1), t, hold["thc1"])
                elif p == 10:
                    emit_hts(1, (2, 3), t, hold["thc1"])
        for p in range(5, NP - 1):
            emit_fc(p, st[p]["ht"], copy_on_act=(p % 2 == 0))
        flush_final([NP - 1])
        emit_fc(NP - 1, st[NP - 1]["ht"], copy_on_act=True)

    nc.compile()
    return nc


def prep_inputs(x, W_ih, W_hh, b_ih, b_hh, W_fc, b_fc, seqs=SEQS, ncores=NCORES):
    """Host-side shard + transpose + weight packing. Returns in_maps."""
    x = np.asarray(x, dtype=np.float32)
    W_ih = np.asarray(W_ih, dtype=np.float32)
    W_hh = np.asarray(W_hh, dtype=np.float32)
    b = np.asarray(b_ih, dtype=np.float32) + np.asarray(b_hh, dtype=np.float32)
    W_fc = np.asarray(W_fc, dtype=np.float32)
    b_fc = np.asarray(b_fc, dtype=np.float32)

    # sections [i, g~(x2), o, f] <- pytorch row groups (i, f, g, o)
    wg = np.zeros((K, G4), dtype=np.float32)
    for dst, src, scale in ((0, 0, 1.0), (H, 2 * H, 2.0), (2 * H, 3 * H, 1.0),
                            (3 * H, H, 1.0)):
        rows = slice(src, src + H)
        wg[0:KH, dst : dst + H] = scale * W_hh[rows, :].T
        wg[KH : KH + C, dst : dst + H] = scale * W_ih[rows, :].T
        wg[K - 1, dst : dst + H] = scale * b[rows]
    wg = wg.astype(NPFP16)

    wfc = np.concatenate([W_fc.T, b_fc[None, :]], axis=0).astype(NPFP16)  # (65, 8)
    bfc = b_fc.reshape(C, 1).astype(np.float32)

    bpc = x.shape[0] // ncores
    in_maps = []
    for k in range(ncores):
        xc = x[k * bpc : (k + 1) * bpc]              # (bpc, T, N, C)
        xt = xc.transpose(3, 1, 0, 2).reshape(C, T, seqs)
        xext = np.empty((KX, T, seqs), dtype=NPFP16)
        xext[0:C] = xt.astype(NPFP16)
        xext[C] = np.ones((T, seqs), dtype=NPFP16)
        in_maps.append({"xin": xext, "wg": wg, "wfc": wfc, "bfc": bfc})
    return in_maps


_CACHE = {}


def _get_nc():
    if "nc" not in _CACHE:
        _CACHE["nc"] = build_nc()
    return _CACHE["nc"]


def kernel(x, W_ih, W_hh, b_ih, b_hh, W_fc, b_fc, **run_kwargs):
    nc = _get_nc()
    in_maps = prep_inputs(x, W_ih, W_hh, b_ih, b_hh, W_fc, b_fc)
    res = run_bass_kernel_spmd(nc, in_maps, list(range(NCORES)), **run_kwargs)
    outs = res.results
    ys = []
    for k in range(NCORES):
        yk = np.asarray(outs[k]["y"])               # (C, SEQS) f32
        ys.append(yk.T.reshape(BPC, N, C))
    y = np.concatenate(ys, axis=0)                  # (B, N, C)
    if run_kwargs.get("trace"):
        _CACHE["last_result"] = res
    return y.astype(np.float32)
